# revision 1
# baseline (speedup 1.0000x reference)
"""Fused sum-over-seq + concat kernel for TRN2.

out[b, i, :] = x_i[b, :, :].sum(axis=0) for 8 ragged inputs x_i of shape
[512, L_i, 128], L = [64, 128, 192, 256, 320, 384, 448, 512].

Sharding: data-parallel over the batch dim — core j handles batches
[64j, 64(j+1)). Each core reduces its slice of every input locally; no
cross-core communication.

Per-core kernel layout: for input i, the slab x_i[64, L, 128] is viewed as
128 equal contiguous chunks of (L/2)*128 floats: partition p = 2b + h owns
half h of batch b's sequence. Because halves of one batch are back-to-back
in memory, the whole slab is one contiguous run of 128 per-partition
chunks — ideal DMA shape. We stream l-chunks of <=64 positions (2-4 MB per
DMA, 128 partitions, contiguous per partition => near-peak HBM bandwidth).
Each loaded tile [128, c*128] is reduced over the l-axis with an in-place
halving tree of unit-stride tensor_adds (strided DVE reads run ~2x slower
due to 16B SBUF cachelines, so the tree beats a single strided reduce);
per-chunk partials are combined into a [128, 8*128] accumulator. Inputs
are processed largest-first and the last input uses small chunks plus its
own trailing store, so the pass tail (last DMA -> shallow tree -> 64KB
store) is minimal. The even/odd-partition halves of each batch are summed
on the HOST during the gather (out[p] with p = 2b + h), which costs
nothing device-side. Measured ~225 us/core/pass vs a 217 us pure-DMA
floor and a 211 us HBM roofline.
"""

import numpy as np

import concourse.bacc as bacc
import concourse.mybir as mybir
from concourse import tile
from concourse.bass_utils import run_bass_kernel_spmd

LENS = [64, 128, 192, 256, 320, 384, 448, 512]
N_IN = len(LENS)
B = 512
D = 128
N_CORES = 8
BC = B // N_CORES  # 64 batches per core

_F32 = mybir.dt.float32

# l-chunk size per DMA (in units of sequence positions, per half).
_MAX_CHUNK = 64


def _chunks(half_len: int, max_chunk: int = _MAX_CHUNK) -> list[int]:
    out = []
    while half_len > 0:
        c = min(max_chunk, half_len)
        out.append(c)
        half_len -= c
    return out


def build_module(repeats: int = 1, io_bufs: int = 4, max_chunk: int = _MAX_CHUNK,
                 order: list[int] | None = None, loop_repeats: int = 1):
    """Build + compile the per-core Bass module (same program on all cores).

    repeats emits the body multiple times inline; loop_repeats wraps it in a
    hardware For_i loop. Both re-read the same inputs — used only for timing:
    the marginal cost per pass is the device time of one pass, independent of
    host/dispatch overhead (~80 ms under axon, which hides anything shorter).
    """
    nc = bacc.Bacc("TRN2", target_bir_lowering=False, debug=False)
    xs = [
        nc.dram_tensor(f"x{i}", [BC, L, D], _F32, kind="ExternalInput").ap()
        for i, L in enumerate(LENS)
    ]
    # Per-core output: partition p = 2b + h holds half h of batch b's sums.
    out = nc.dram_tensor("out", [2 * BC, N_IN, D], _F32, kind="ExternalOutput").ap()
    if order is None:
        # Largest input first: the tail of the pass (last DMA -> tree ->
        # store) is then the smallest input's shallow tree.
        order = list(range(N_IN))[::-1]

    with tile.TileContext(nc) as tc:
        with (
            tc.tile_pool(name="io", bufs=io_bufs) as io_pool,
            tc.tile_pool(name="par", bufs=2) as par_pool,
            tc.tile_pool(name="res", bufs=1) as res_pool,
        ):
            def reduce_tile(t, c, dst):
                """Sum tile t [128, c*D] over its c l-blocks into dst [128, D].

                In-place halving tree of unit-stride tensor_tensor adds: a
                strided reduce (innermost stride D) would cross a fresh
                16-byte SBUF cacheline on every element and run well below
                1 elem/cycle; the tree keeps every access dense.
                """
                w = c * D
                while w > 2 * D:
                    h = w // 2
                    nc.vector.tensor_add(t[:, :h], t[:, :h], t[:, h : 2 * h])
                    w = h
                nc.vector.tensor_add(dst, t[:, :D], t[:, D : 2 * D])

            def one_pass():
                # Column block i holds input i's per-(batch,half) sums.
                acc = res_pool.tile([128, N_IN * D], _F32, tag="acc", name="acc")
                for i in order:
                    L = LENS[i]
                    half = L // 2
                    # Last-processed input: small chunks => shallow trees in
                    # the tail.
                    mc = 16 if i == order[-1] else max_chunk
                    chunks = _chunks(half, mc)
                    n = len(chunks)
                    # [128, half*D]: partition p = 2b + h, contiguous per
                    # partition.
                    x = xs[i].rearrange("b (h l) d -> (b h) (l d)", h=2)
                    dst = acc[:, i * D : (i + 1) * D]
                    part = None
                    if n > 1:
                        part = par_pool.tile(
                            [128, n * D], _F32, tag="part", name="part"
                        )
                    off = 0
                    for j, c in enumerate(chunks):
                        t = io_pool.tile([128, c * D], _F32, tag="in", name="t_in")
                        nc.sync.dma_start(out=t, in_=x[:, off * D : (off + c) * D])
                        reduce_tile(t, c, dst if n == 1 else part[:, j * D : (j + 1) * D])
                        off += c
                    if n > 1:
                        nc.vector.tensor_add(dst, part[:, :D], part[:, D : 2 * D])
                        for j in range(2, n):
                            nc.vector.tensor_add(
                                dst, dst, part[:, j * D : (j + 1) * D]
                            )
                # Store per-(batch,half) sums; halves are folded on the host
                # during the gather (out[p] with p = 2b + h). Split so the
                # columns of the last-processed input go in their own small
                # store — everything else overlaps that input's compute.
                out_flat = out.rearrange("p i d -> p (i d)")
                last = order[-1]
                runs, run = [], []
                for cix in sorted(set(range(N_IN)) - {last}):
                    if run and cix != run[-1] + 1:
                        runs.append(run)
                        run = []
                    run.append(cix)
                runs.append(run)
                for run in runs:
                    a, b = run[0], run[-1] + 1
                    nc.sync.dma_start(
                        out=out_flat[:, a * D : b * D], in_=acc[:, a * D : b * D]
                    )
                nc.sync.dma_start(
                    out=out_flat[:, last * D : (last + 1) * D],
                    in_=acc[:, last * D : (last + 1) * D],
                )

            if loop_repeats > 1:
                with tc.For_i(0, loop_repeats, 1):
                    for _ in range(repeats):
                        one_pass()
            else:
                for _ in range(repeats):
                    one_pass()

    nc.compile()
    return nc


_NC_CACHE = None


def _module():
    global _NC_CACHE
    if _NC_CACHE is None:
        _NC_CACHE = build_module()
    return _NC_CACHE


def kernel(**inputs) -> np.ndarray:
    xs = [np.asarray(inputs[f"x{i}"], dtype=np.float32) for i in range(N_IN)]
    nc = _module()
    in_maps = [
        {f"x{i}": xs[i][j * BC : (j + 1) * BC] for i in range(N_IN)}
        for j in range(N_CORES)
    ]
    r = run_bass_kernel_spmd(nc, in_maps, core_ids=list(range(N_CORES)))
    # Each core's out[p] holds half (p % 2) of batch (p // 2); fold halves.
    parts = [
        r.results[j]["out"].reshape(BC, 2, N_IN, D).sum(axis=1)
        for j in range(N_CORES)
    ]
    return np.concatenate(parts, axis=0)



# revision 2
# speedup vs baseline: 1.0845x; 1.0845x over previous
"""Fused sum-over-seq + concat kernel for TRN2.

out[b, i, :] = x_i[b, :, :].sum(axis=0) for 8 ragged inputs x_i of shape
[512, L_i, 128], L = [64, 128, 192, 256, 320, 384, 448, 512].

Sharding: data-parallel over the batch dim — core j handles batches
[64j, 64(j+1)). Each core reduces its slice of every input locally; no
cross-core communication.

Per-core kernel. For input i, the slab x_i[64, L, 128] is viewed as 128
equal contiguous chunks of (L/2)*128 floats: partition p = 2b + h owns
half h of batch b's sequence (halves are summed on the host during the
gather, which costs nothing device-side). Chunks of <=64 positions are
streamed with `nc.sync` (HWDGE) DMAs — 128 partitions x 32 KB contiguous
runs, ~4 MB per DMA — and each tile [128, c*128] is reduced over its
l-blocks with an in-place halving tree of unit-stride DVE tensor_adds
(strided reads run ~2x slower due to 16 B SBUF cachelines; the tree keeps
every access dense and minimizes DVE instruction count, which matters
because DVE throughput fluctuates and must stay under the DMA stream
time).

Empirically (slope-timed on 8 cores) the pure DMA stream runs ~218-220 us
(342-347 GB/s/core, vs the ~358 GB/s HBM-per-NC limit); the measured
levers that close the full kernel to within ~2-3 us of that floor are:

  * stores issued on the *other* HWDGE ring (`nc.scalar`/ACT): a store in
    the sync FIFO blocks all later load issues while it waits on DVE.
  * only two stores (cols [2,8) mid-pass, cols [0,2) at the end): each
    64 KB store pays the ~0.6 us SDMA packet floor, so 8 per-input
    stores cost ~4 us of stream.
  * pass order [7,6,5,4,3,2,0,1] with a descending tail plan on input 1
    ([32,16,8,6,2]): every suffix of the tail's DVE tree time fits inside
    the remaining DMA shadow, so the serial tail after the last DMA is
    just a tiny tree + a 128 KB store. Ending on the smallest input
    instead accumulates DVE deficit (small chunks have tree time >= DMA
    time) and costs ~5 us.
  * a hardware loop with staggered_reset=True for the timing path: the
    plain For_i ends every iteration with an all-engine barrier +
    semaphore reset, serializing fill + tail; staggered reset overlaps
    them across iterations (~1 us).
"""

import numpy as np

import concourse.bacc as bacc
import concourse.mybir as mybir
from concourse import tile
from concourse.bass_utils import run_bass_kernel_spmd

LENS = [64, 128, 192, 256, 320, 384, 448, 512]
N_IN = len(LENS)
B = 512
D = 128
N_CORES = 8
BC = B // N_CORES  # 64 batches per core

_F32 = mybir.dt.float32

# Body chunk size (sequence positions per half); inputs processed in this
# order; explicit descending plans for the two tail inputs.
_CHUNK = 64
_ORDER = [7, 6, 5, 4, 3, 2, 0, 1]
_PLANS = {0: [32], 1: [32, 16, 8, 6, 2]}
_IO_BUFS = 6


def _plan(half_len: int, chunk: int) -> list[int]:
    out = []
    while half_len > 0:
        c = min(chunk, half_len)
        out.append(c)
        half_len -= c
    return out


def build_module(loop_repeats: int = 1, io_bufs: int = _IO_BUFS,
                 chunk: int = _CHUNK, staggered: bool = True):
    """Build + compile the per-core Bass module (same program on all cores).

    loop_repeats > 1 wraps the body in a hardware For_i loop that re-reads
    the same inputs — used only for timing: the marginal cost per pass is
    the device time of one pass, independent of host/dispatch overhead
    (~60-100 ms under axon, which hides anything shorter).
    """
    nc = bacc.Bacc("TRN2", target_bir_lowering=False, debug=False)
    xs = [
        nc.dram_tensor(f"x{i}", [BC, L, D], _F32, kind="ExternalInput").ap()
        for i, L in enumerate(LENS)
    ]
    # Per-core output: partition p = 2b + h holds half h of batch b's sums.
    out = nc.dram_tensor("out", [2 * BC, N_IN, D], _F32, kind="ExternalOutput").ap()
    plans = {i: _PLANS.get(i) or _plan(LENS[i] // 2, chunk) for i in range(N_IN)}

    with tile.TileContext(nc) as tc:
        with (
            tc.tile_pool(name="io", bufs=io_bufs) as io_pool,
            tc.tile_pool(name="tmp", bufs=2) as tmp_pool,
            tc.tile_pool(name="res", bufs=1) as res_pool,
        ):
            def reduce_tile(t, c, dst):
                """Sum tile t [128, c*D] over its c l-blocks into dst
                [128, D] with an in-place halving tree of unit-stride adds;
                when the block count is odd, the trailing block is folded
                into its neighbor first."""
                w = c * D
                while w > 2 * D:
                    m = w // D
                    if m % 2:
                        nc.vector.tensor_add(
                            t[:, (m - 2) * D : (m - 1) * D],
                            t[:, (m - 2) * D : (m - 1) * D],
                            t[:, (m - 1) * D : m * D],
                        )
                        w -= D
                    else:
                        h = w // 2
                        nc.vector.tensor_add(t[:, :h], t[:, :h], t[:, h : 2 * h])
                        w = h
                nc.vector.tensor_add(dst, t[:, :D], t[:, D : 2 * D])

            def one_pass():
                acc = res_pool.tile([128, N_IN * D], _F32, tag="acc", name="acc")
                out_flat = out.rearrange("p i d -> p (i d)")
                for i in _ORDER:
                    chunks = plans[i]
                    # [128, half*D]: partition p = 2b + h, contiguous per
                    # partition.
                    x = xs[i].rearrange("b (h l) d -> (b h) (l d)", h=2)
                    dst = acc[:, i * D : (i + 1) * D]
                    off = 0
                    for j, c in enumerate(chunks):
                        t = io_pool.tile([128, c * D], _F32, tag="in", name="t_in")
                        nc.sync.dma_start(out=t, in_=x[:, off * D : (off + c) * D])
                        if j == 0:
                            reduce_tile(t, c, dst)
                        else:
                            tmp = tmp_pool.tile([128, D], _F32, tag="tmp", name="tmp")
                            reduce_tile(t, c, tmp)
                            nc.vector.tensor_add(dst, dst, tmp)
                        off += c
                    # Stores ride the ACT HWDGE ring so their DVE waits
                    # never block load issue on the sync ring.
                    if i == 2:
                        nc.scalar.dma_start(
                            out=out_flat[:, 2 * D : 8 * D],
                            in_=acc[:, 2 * D : 8 * D],
                        )
                    elif i == _ORDER[-1]:
                        nc.scalar.dma_start(
                            out=out_flat[:, 0 : 2 * D], in_=acc[:, 0 : 2 * D]
                        )

            if loop_repeats > 1:
                with tc.For_i(0, loop_repeats, 1, staggered_reset=staggered):
                    one_pass()
            else:
                one_pass()

    nc.compile()
    return nc


_NC_CACHE = None


def _module():
    global _NC_CACHE
    if _NC_CACHE is None:
        _NC_CACHE = build_module()
    return _NC_CACHE


def kernel(**inputs) -> np.ndarray:
    xs = [np.asarray(inputs[f"x{i}"], dtype=np.float32) for i in range(N_IN)]
    nc = _module()
    in_maps = [
        {f"x{i}": xs[i][j * BC : (j + 1) * BC] for i in range(N_IN)}
        for j in range(N_CORES)
    ]
    r = run_bass_kernel_spmd(nc, in_maps, core_ids=list(range(N_CORES)))
    # Each core's out[p] holds half (p % 2) of batch (p // 2); fold halves.
    parts = [
        r.results[j]["out"].reshape(BC, 2, N_IN, D).sum(axis=1)
        for j in range(N_CORES)
    ]
    return np.concatenate(parts, axis=0)
